# revision 4
# baseline (speedup 1.0000x reference)
"""Trainium2 Bass kernel v3 for quantized-MLP-with-LoRA.

Data-parallel over tokens (1024/core). v3 feeds the device with host-side
LAYOUT-transformed inputs (pure transpose/reshape/replication — all
arithmetic stays on device):
- quantized codes pre-tiled so each h-tile (up) / d-tile (down) loads as one
  contiguous [128, ...] block already in lhsT orientation;
- per-block scales pre-expanded (np.repeat) and pre-tiled to the same layout
  (f16), so dequant is two dense DVE ops and the result is directly the
  matmul stationary operand — no on-chip transposes in the hot loops.
x2 (hidden activation) stays fully SBUF-resident; y2 is produced in [d, tok]
layout and un-transposed on the host while gathering.
"""
import sys

if "/opt/trn_rl_repo" not in sys.path:
    sys.path.insert(0, "/opt/trn_rl_repo")

import numpy as np

import concourse.bass as bass
import concourse.mybir as mybir
import concourse.tile as tile
from concourse import bacc
from concourse.bass import ts, ds
from concourse.bass_utils import run_bass_kernel_spmd

F16 = mybir.dt.float16
F32 = mybir.dt.float32
I32 = mybir.dt.int32

NCORES = 8
T = 1024          # tokens per core
D = 2048
H = 8192
R = 16
P = 128
KD = D // P       # 16 contraction tiles for up
KH = H // P       # 64 contraction tiles for down
DT = D // P       # 16 output d-tiles for down
NT = T // 512     # 2 moving-operand chunks of 512 tokens

TRACE = False
LAST_RESULTS = None


def _build():
    nc = bacc.Bacc("TRN2", target_bir_lowering=False, debug=False,
                   enable_asserts=False, num_devices=NCORES)

    x1c = nc.dram_tensor("x1c", [T, D], F32, kind="ExternalInput").ap()
    wupL = nc.dram_tensor("wupL", [KH, P, D], I32, kind="ExternalInput").ap()
    supL = nc.dram_tensor("supL", [KH, P, D], F16, kind="ExternalInput").ap()
    bup = nc.dram_tensor("bup", [H], F32, kind="ExternalInput").ap()
    a1 = nc.dram_tensor("a1", [D, R], F32, kind="ExternalInput").ap()
    b1 = nc.dram_tensor("b1", [R, H], F32, kind="ExternalInput").ap()
    wdnL = nc.dram_tensor("wdnL", [DT, P, H], I32, kind="ExternalInput").ap()
    sdnL = nc.dram_tensor("sdnL", [DT, P, H], F16, kind="ExternalInput").ap()
    bdn = nc.dram_tensor("bdn", [D], F32, kind="ExternalInput").ap()
    a2 = nc.dram_tensor("a2", [H, R], F32, kind="ExternalInput").ap()
    b2 = nc.dram_tensor("b2", [R, D], F32, kind="ExternalInput").ap()
    y2t = nc.dram_tensor("y2t", [D, T], F32, kind="ExternalOutput").ap()

    with tile.TileContext(nc) as tc:
        with tc.tile_pool(name="big", bufs=1) as bp, \
             tc.tile_pool(name="const", bufs=1) as cp, \
             tc.tile_pool(name="psum", bufs=6, space="PSUM") as pp, \
             tc.tile_pool(name="psum_vt", bufs=1, space="PSUM") as pvt:

            # resident hidden activation: x2sb[h_part, kh, tok]
            x2sb = bp.tile([P, KH, T], F16, tag="x2sb")

            # constants needed by both phases
            a2f = cp.tile([P, KH, R], F16, tag="a2f")
            v1t = cp.tile([R + 1, T], F16, tag="v1t")
            # row R of v1t stays 1.0 → folds b_down into the lora matmul
            nc.any.memset(v1t[:], 1.0)

            vt_ps = [pvt.tile([R, 512], F32, tag=f"vt{i}", name=f"vt{i}")
                     for i in range(NT)]

            # ================= UP phase =================
            with tc.tile_pool(name="upc", bufs=1) as up, \
                 tc.tile_pool(name="b1p", bufs=2) as b1pool, \
                 tc.tile_pool(name="qstp", bufs=2) as qp, \
                 tc.tile_pool(name="sflp", bufs=2) as sfp, \
                 tc.tile_pool(name="qfp", bufs=3) as qfp:

                x1t = up.tile([P, KD, T], F16, tag="x1t")
                bupsb = up.tile([P, KH], F32, tag="bupsb")
                utf = up.tile([R, T], F16, tag="utf")

                # ---- prep constants (cast during SWDGE DMA) ----
                nc.gpsimd.dma_start(a2f[:], a2.rearrange("(o p) r -> p o r", p=P))
                a1f = up.tile([P, KD, R], F16, tag="a1f")
                nc.gpsimd.dma_start(a1f[:], a1.rearrange("(o p) r -> p o r", p=P))
                nc.sync.dma_start(bupsb[:], bup.rearrange("(o p) -> p o", p=P))

                # ---- x1 -> fp16 (cast during SWDGE DMA), transpose ----
                for s in range(T // P):
                    xf = qfp.tile([P, D], F16, tag="qf", name=f"xf{s}")
                    nc.gpsimd.dma_start(xf[:], x1c[ts(s, P), :])
                    nc.scalar.dma_start_transpose(x1t[:, :, ts(s, P)], xf[:])

                # ---- uT = (x1 @ A1)^T : [R, T] ----
                for tt in range(NT):
                    ups = pp.tile([R, 512], F32, tag="mm")
                    for j in range(KD):
                        nc.tensor.matmul(ups[:], a1f[:, j, :], x1t[:, j, ts(tt, 512)],
                                         start=(j == 0), stop=(j == KD - 1))
                    nc.scalar.copy(utf[:, ts(tt, 512)], ups[:])

                # ---- main up loop: one 128-row slab of H per step ----
                # software pipeline: loads k+2, dequant k+1, matmuls k
                def st_load(k):
                    qsts = []
                    for hh in range(2):
                        qst = qp.tile([P, D // 2], I32, tag="qst", bufs=4,
                                      name=f"qst{k}_{hh}")
                        nc.sync.dma_start(qst[:], wupL[k, :, ts(hh, D // 2)])
                        qsts.append(qst)
                    sfl = sfp.tile([P, D], F16, tag="sfl", bufs=2,
                                   name=f"sfl{k}")
                    nc.sync.dma_start(sfl[:], supL[k])
                    b1st = b1pool.tile([R, P], F32, tag="b1st", bufs=3,
                                       name=f"b1st{k}")
                    nc.sync.dma_start(b1st[:], b1[:, ts(k, P)])
                    return qsts, sfl, b1st

                def st_dequant(k, loaded):
                    qsts, sfl, b1st = loaded
                    b1sl = b1pool.tile([R, P], F16, tag="b1sl", bufs=3,
                                       name=f"b1sl{k}")
                    nc.scalar.copy(b1sl[:], b1st[:])
                    qf = qfp.tile([P, D], F16, tag="qf", name=f"qf{k}")
                    for hh in range(2):
                        nc.vector.tensor_scalar_add(qf[:, ts(hh, D // 2)],
                                                    qsts[hh][:], -7.5)
                    nc.vector.tensor_tensor(qf[:], qf[:], sfl[:],
                                            mybir.AluOpType.mult)
                    return qf, b1sl

                LD, DQ = {}, {}
                for kk in range(2):
                    LD[kk] = st_load(kk)
                DQ[0] = st_dequant(0, LD.pop(0))

                for k in range(KH):
                    if k + 2 < KH:
                        LD[k + 2] = st_load(k + 2)
                    if k + 1 < KH:
                        DQ[k + 1] = st_dequant(k + 1, LD.pop(k + 1))
                    qf, b1sl = DQ.pop(k)
                    wt = qf.rearrange("p (j h) -> p j h", h=P)

                    # PE order: G(k,0); deferred vt(k-1,1); G(k,1); vt(k,0).
                    for tt in range(NT):
                        ps = pp.tile([P, 512], F32, tag="mm")
                        for j in range(KD):
                            nc.tensor.matmul(ps[:], wt[:, j, :], x1t[:, j, ts(tt, 512)],
                                             start=(j == 0), stop=False)
                        nc.tensor.matmul(ps[:], b1sl[:], utf[:, ts(tt, 512)],
                                         start=False, stop=True)
                        nc.scalar.activation(x2sb[:, k, ts(tt, 512)], ps[:],
                                             mybir.ActivationFunctionType.Relu,
                                             bias=bupsb[:, k:k + 1], scale=1.0)
                        if tt == 0 and k > 0:
                            nc.tensor.matmul(vt_ps[1][:], a2f[:, k - 1, :],
                                             x2sb[:, k - 1, ds(512, 512)],
                                             start=(k - 1 == 0), stop=False,
                                             skip_group_check=True)
                    nc.tensor.matmul(vt_ps[0][:], a2f[:, k, :],
                                     x2sb[:, k, ds(0, 512)],
                                     start=(k == 0), stop=(k == KH - 1),
                                     skip_group_check=True)
                nc.tensor.matmul(vt_ps[1][:], a2f[:, KH - 1, :],
                                 x2sb[:, KH - 1, ds(512, 512)],
                                 start=False, stop=True,
                                 skip_group_check=True)

                for tt in range(NT):
                    nc.scalar.copy(v1t[:R, ts(tt, 512)], vt_ps[tt][:])

            # ================= DOWN phase =================
            with tc.tile_pool(name="dconst", bufs=1) as dcp, \
                 tc.tile_pool(name="dstage", bufs=2) as dsp, \
                 tc.tile_pool(name="dsfl", bufs=2) as dsf, \
                 tc.tile_pool(name="wdn", bufs=2) as wd, \
                 tc.tile_pool(name="yout", bufs=3) as yp:
                b2p = dcp.tile([R + 1, D], F16, tag="b2p")
                for hf in range(2):
                    stx = dsp.tile([P, D // 2], F32, tag="dst32", bufs=1)
                    nc.sync.dma_start(stx[:R, :], b2[:, ts(hf, D // 2)])
                    nc.sync.dma_start(stx[R:R + 1, :], bdn[None, ts(hf, D // 2)])
                    nc.vector.tensor_copy(b2p[:, ts(hf, D // 2)], stx[:R + 1, :])

                def dn_dequant(dt):
                    wdf = wd.tile([P, KH, P], F16, tag="wdf")
                    for c in range(4):            # 16-kh-tile chunks
                        sfd = dsf.tile([P, D], F16, tag="sfd", bufs=2,
                                       name=f"sfd{dt}_{c}")
                        nc.sync.dma_start(sfd[:], sdnL[dt, :, ts(c, D)])
                        for hh in range(2):
                            qst = dsp.tile([P, D // 2], I32, tag="qst", bufs=4,
                                           name=f"dq{dt}_{c}_{hh}")
                            nc.sync.dma_start(
                                qst[:], wdnL[dt, :, ds(c * D + hh * (D // 2), D // 2)])
                            nc.vector.tensor_scalar_add(
                                wdf[:].rearrange("p j h -> p (j h)")[
                                    :, ds(c * D + hh * (D // 2), D // 2)],
                                qst[:], -7.5)
                        v = wdf[:].rearrange("p j h -> p (j h)")[:, ts(c, D)]
                        nc.vector.tensor_tensor(v, v, sfd[:], mybir.AluOpType.mult)
                    return wdf

                dpipe = {0: dn_dequant(0)}
                for dt in range(DT):
                    if dt + 1 < DT:
                        dpipe[dt + 1] = dn_dequant(dt + 1)
                    wdf = dpipe.pop(dt)

                    for tt in range(NT):
                        ps = pp.tile([P, 512], F32, tag="mm")
                        for k in range(KH):
                            nc.tensor.matmul(ps[:], wdf[:, k, :],
                                             x2sb[:, k, ts(tt, 512)],
                                             start=(k == 0), stop=False)
                        nc.tensor.matmul(ps[:], b2p[:, ts(dt, P)], v1t[:, ts(tt, 512)],
                                         start=False, stop=True)
                        yo = yp.tile([P, 512], F32, tag="yo")
                        nc.scalar.copy(yo[:], ps[:])
                        nc.sync.dma_start(y2t[ts(dt, P), ts(tt, 512)], yo[:])

    nc.compile()
    return nc


_NC = None


def build_in_maps(inputs):
    x1 = np.ascontiguousarray(np.asarray(inputs["x1"], dtype=np.float32))
    B, S, _ = x1.shape
    xf = x1.reshape(B * S, D)

    wq = np.asarray(inputs["w_up_q"], dtype=np.int32)        # [H, D]
    sup = np.asarray(inputs["w_up_scale"], dtype=np.float32)  # [H, 32]
    # codes, lhsT-tiled: wupL[k][p, j*128+h] = w_up_q[k*128+h, j*128+p]
    wupL = np.ascontiguousarray(
        wq.T.reshape(KD, P, KH, P).transpose(2, 1, 0, 3).reshape(KH, P, D))
    # scales expanded to the same layout (replication only, f16)
    sfull = np.repeat(sup.T, 64, axis=0)                      # [D, H]
    supL = np.ascontiguousarray(
        sfull.reshape(KD, P, KH, P).transpose(2, 1, 0, 3).reshape(KH, P, D)
    ).astype(np.float16)

    wdq = np.asarray(inputs["w_down_q"], dtype=np.int32)      # [D, H]
    sdn = np.asarray(inputs["w_down_scale"], dtype=np.float32)  # [D, 128]
    # codes, lhsT-tiled: wdnL[dt][p, k*128+d] = w_down_q[dt*128+d, k*128+p]
    wdnL = np.ascontiguousarray(
        wdq.T.reshape(KH, P, DT, P).transpose(2, 1, 0, 3).reshape(DT, P, H))
    sdfull = np.repeat(sdn.T, 64, axis=0)                     # [H, D]
    sdnL = np.ascontiguousarray(
        sdfull.reshape(KH, P, DT, P).transpose(2, 1, 0, 3).reshape(DT, P, H)
    ).astype(np.float16)

    shared = {
        "wupL": wupL, "supL": supL,
        "bup": np.ascontiguousarray(np.asarray(inputs["b_up"], dtype=np.float32)),
        "a1": np.ascontiguousarray(np.asarray(inputs["w_up_lora_a"], dtype=np.float32)),
        "b1": np.ascontiguousarray(np.asarray(inputs["w_up_lora_b"], dtype=np.float32)),
        "wdnL": wdnL, "sdnL": sdnL,
        "bdn": np.ascontiguousarray(np.asarray(inputs["b_down"], dtype=np.float32)),
        "a2": np.ascontiguousarray(np.asarray(inputs["w_down_lora_a"], dtype=np.float32)),
        "b2": np.ascontiguousarray(np.asarray(inputs["w_down_lora_b"], dtype=np.float32)),
    }
    return [{"x1c": np.ascontiguousarray(xf[c * T:(c + 1) * T]), **shared}
            for c in range(NCORES)]


def kernel(x1, w_up_q, w_up_scale, b_up, w_up_lora_a, w_up_lora_b,
           w_down_q, w_down_scale, b_down, w_down_lora_a, w_down_lora_b):
    global _NC, LAST_RESULTS
    if _NC is None:
        _NC = _build()

    inputs = dict(x1=x1, w_up_q=w_up_q, w_up_scale=w_up_scale, b_up=b_up,
                  w_up_lora_a=w_up_lora_a, w_up_lora_b=w_up_lora_b,
                  w_down_q=w_down_q, w_down_scale=w_down_scale, b_down=b_down,
                  w_down_lora_a=w_down_lora_a, w_down_lora_b=w_down_lora_b)
    in_maps = build_in_maps(inputs)
    res = run_bass_kernel_spmd(_NC, in_maps, core_ids=list(range(NCORES)),
                               trace=TRACE)
    LAST_RESULTS = res
    B, S, _ = np.asarray(x1).shape
    # y2t is [D, T] per core — un-transpose on the host while gathering
    out = np.concatenate([res.results[c]["y2t"].T for c in range(NCORES)], axis=0)
    return np.ascontiguousarray(out).reshape(B, S, D)


# revision 6
# speedup vs baseline: 1.2350x; 1.2350x over previous
"""Trainium2 Bass kernel v3 for quantized-MLP-with-LoRA.

Data-parallel over tokens (1024/core). v3 feeds the device with host-side
LAYOUT-transformed inputs (pure transpose/reshape/replication — all
arithmetic stays on device):
- quantized codes pre-tiled so each h-tile (up) / d-tile (down) loads as one
  contiguous [128, ...] block already in lhsT orientation;
- per-block scales pre-expanded (np.repeat) and pre-tiled to the same layout
  (f16), so dequant is two dense DVE ops and the result is directly the
  matmul stationary operand — no on-chip transposes in the hot loops.
x2 (hidden activation) stays fully SBUF-resident; y2 is produced in [d, tok]
layout and un-transposed on the host while gathering.
"""
import sys

if "/opt/trn_rl_repo" not in sys.path:
    sys.path.insert(0, "/opt/trn_rl_repo")

import numpy as np

import concourse.bass as bass
import concourse.mybir as mybir
import concourse.tile as tile
from concourse import bacc
from concourse.bass import ts, ds
from concourse.bass_utils import run_bass_kernel_spmd

F16 = mybir.dt.float16
F32 = mybir.dt.float32
I32 = mybir.dt.int32

NCORES = 8
T = 1024          # tokens per core
D = 2048
H = 8192
R = 16
P = 128
KD = D // P       # 16 contraction tiles for up
KH = H // P       # 64 contraction tiles for down
DT = D // P       # 16 output d-tiles for down
NT = T // 512     # 2 moving-operand chunks of 512 tokens

TRACE = False
LAST_RESULTS = None


def _build():
    nc = bacc.Bacc("TRN2", target_bir_lowering=False, debug=False,
                   enable_asserts=False, num_devices=NCORES)

    x1c = nc.dram_tensor("x1c", [T, D], F32, kind="ExternalInput").ap()
    wupL = nc.dram_tensor("wupL", [KH, P, D], I32, kind="ExternalInput").ap()
    supL = nc.dram_tensor("supL", [KH, P, D], F16, kind="ExternalInput").ap()
    bup = nc.dram_tensor("bup", [H], F32, kind="ExternalInput").ap()
    a1 = nc.dram_tensor("a1", [D, R], F32, kind="ExternalInput").ap()
    b1 = nc.dram_tensor("b1", [R, H], F32, kind="ExternalInput").ap()
    wdnL = nc.dram_tensor("wdnL", [DT, P, H], I32, kind="ExternalInput").ap()
    sdnL = nc.dram_tensor("sdnL", [DT, P, H], F16, kind="ExternalInput").ap()
    bdn = nc.dram_tensor("bdn", [D], F32, kind="ExternalInput").ap()
    a2 = nc.dram_tensor("a2", [H, R], F32, kind="ExternalInput").ap()
    b2 = nc.dram_tensor("b2", [R, D], F32, kind="ExternalInput").ap()
    y2t = nc.dram_tensor("y2t", [D, T], F32, kind="ExternalOutput").ap()

    with tile.TileContext(nc) as tc:
        with tc.tile_pool(name="big", bufs=1) as bp, \
             tc.tile_pool(name="const", bufs=1) as cp, \
             tc.tile_pool(name="psum", bufs=6, space="PSUM") as pp, \
             tc.tile_pool(name="psum_vt", bufs=1, space="PSUM") as pvt:

            # resident hidden activation: x2sb[h_part, kh, tok]
            x2sb = bp.tile([P, KH, T], F16, tag="x2sb")

            # constants needed by both phases
            a2f = cp.tile([P, KH, R], F16, tag="a2f")
            v1t = cp.tile([R + 1, T], F16, tag="v1t")
            # row R of v1t stays 1.0 → folds b_down into the lora matmul
            nc.any.memset(v1t[:], 1.0)

            vt_ps = [pvt.tile([R, 512], F32, tag=f"vt{i}", name=f"vt{i}")
                     for i in range(NT)]

            # ================= UP phase =================
            with tc.tile_pool(name="upc", bufs=1) as up, \
                 tc.tile_pool(name="b1p", bufs=2) as b1pool, \
                 tc.tile_pool(name="qstp", bufs=2) as qp, \
                 tc.tile_pool(name="sflp", bufs=2) as sfp, \
                 tc.tile_pool(name="qfp", bufs=3) as qfp:

                x1t = up.tile([P, KD, T], F16, tag="x1t")
                bupsb = up.tile([P, KH], F32, tag="bupsb")
                utf = up.tile([R, T], F16, tag="utf")

                # ---- prep constants (cast during SWDGE DMA) ----
                nc.gpsimd.dma_start(a2f[:], a2.rearrange("(o p) r -> p o r", p=P))
                a1f = up.tile([P, KD, R], F16, tag="a1f")
                nc.gpsimd.dma_start(a1f[:], a1.rearrange("(o p) r -> p o r", p=P))
                nc.sync.dma_start(bupsb[:], bup.rearrange("(o p) -> p o", p=P))

                # ---- x1 -> fp16 (cast during SWDGE DMA), transpose ----
                for s in range(T // P):
                    xf = qfp.tile([P, D], F16, tag="qf", bufs=4, name=f"xf{s}")
                    nc.gpsimd.dma_start(xf[:], x1c[ts(s, P), :])
                    nc.scalar.dma_start_transpose(x1t[:, :, ts(s, P)], xf[:])

                # ---- uT = (x1 @ A1)^T : [R, T] ----
                for tt in range(NT):
                    ups = pp.tile([R, 512], F32, tag="mm")
                    for j in range(KD):
                        nc.tensor.matmul(ups[:], a1f[:, j, :], x1t[:, j, ts(tt, 512)],
                                         start=(j == 0), stop=(j == KD - 1))
                    nc.scalar.copy(utf[:, ts(tt, 512)], ups[:])

                # ---- main up loop: one 128-row slab of H per step ----
                # software pipeline: loads k+2, dequant k+1, matmuls k.
                # codes are cast i32->f16 during the SWDGE DMA (0..15 exact),
                # so dequant is an in-place 4x-mode add plus one dense mult.
                def st_load(k):
                    qf = qfp.tile([P, D], F16, tag="qf", bufs=4, name=f"qf{k}")
                    nc.gpsimd.dma_start(qf[:], wupL[k])
                    sfl = sfp.tile([P, D], F16, tag="sfl", bufs=3,
                                   name=f"sfl{k}")
                    nc.sync.dma_start(sfl[:], supL[k])
                    b1st = b1pool.tile([R, P], F32, tag="b1st", bufs=3,
                                       name=f"b1st{k}")
                    nc.sync.dma_start(b1st[:], b1[:, ts(k, P)])
                    return qf, sfl, b1st

                def st_dequant(k, loaded):
                    qf, sfl, b1st = loaded
                    b1sl = b1pool.tile([R, P], F16, tag="b1sl", bufs=3,
                                       name=f"b1sl{k}")
                    nc.scalar.copy(b1sl[:], b1st[:])
                    nc.vector.tensor_scalar_add(qf[:], qf[:], -7.5)
                    nc.vector.tensor_tensor(qf[:], qf[:], sfl[:],
                                            mybir.AluOpType.mult)
                    return qf, b1sl

                LD, DQ = {}, {}
                for kk in range(2):
                    LD[kk] = st_load(kk)
                DQ[0] = st_dequant(0, LD.pop(0))

                for k in range(KH):
                    if k + 2 < KH:
                        LD[k + 2] = st_load(k + 2)
                    if k + 1 < KH:
                        DQ[k + 1] = st_dequant(k + 1, LD.pop(k + 1))
                    qf, b1sl = DQ.pop(k)
                    wt = qf.rearrange("p (j h) -> p j h", h=P)

                    # PE order: G(k,0); deferred vt(k-1,1); G(k,1); vt(k,0).
                    for tt in range(NT):
                        ps = pp.tile([P, 512], F32, tag="mm")
                        for j in range(KD):
                            nc.tensor.matmul(ps[:], wt[:, j, :], x1t[:, j, ts(tt, 512)],
                                             start=(j == 0), stop=False)
                        nc.tensor.matmul(ps[:], b1sl[:], utf[:, ts(tt, 512)],
                                         start=False, stop=True)
                        nc.scalar.activation(x2sb[:, k, ts(tt, 512)], ps[:],
                                             mybir.ActivationFunctionType.Relu,
                                             bias=bupsb[:, k:k + 1], scale=1.0)
                        if tt == 0 and k > 0:
                            nc.tensor.matmul(vt_ps[1][:], a2f[:, k - 1, :],
                                             x2sb[:, k - 1, ds(512, 512)],
                                             start=(k - 1 == 0), stop=False,
                                             skip_group_check=True)
                    nc.tensor.matmul(vt_ps[0][:], a2f[:, k, :],
                                     x2sb[:, k, ds(0, 512)],
                                     start=(k == 0), stop=(k == KH - 1),
                                     skip_group_check=True)
                nc.tensor.matmul(vt_ps[1][:], a2f[:, KH - 1, :],
                                 x2sb[:, KH - 1, ds(512, 512)],
                                 start=False, stop=True,
                                 skip_group_check=True)

                for tt in range(NT):
                    nc.scalar.copy(v1t[:R, ts(tt, 512)], vt_ps[tt][:])

            # ================= DOWN phase =================
            with tc.tile_pool(name="dconst", bufs=1) as dcp, \
                 tc.tile_pool(name="dstage", bufs=2) as dsp, \
                 tc.tile_pool(name="dsfl", bufs=2) as dsf, \
                 tc.tile_pool(name="wdn", bufs=2) as wd, \
                 tc.tile_pool(name="yout", bufs=3) as yp:
                b2p = dcp.tile([R + 1, D], F16, tag="b2p")
                for hf in range(2):
                    stx = dsp.tile([P, D // 2], F32, tag="dst32", bufs=1)
                    nc.sync.dma_start(stx[:R, :], b2[:, ts(hf, D // 2)])
                    nc.sync.dma_start(stx[R:R + 1, :], bdn[None, ts(hf, D // 2)])
                    nc.vector.tensor_copy(b2p[:, ts(hf, D // 2)], stx[:R + 1, :])

                def dn_dequant(dt):
                    wdf = wd.tile([P, KH, P], F16, tag="wdf")
                    for c in range(4):            # 16-kh-tile chunks
                        sfd = dsf.tile([P, D], F16, tag="sfd", bufs=3,
                                       name=f"sfd{dt}_{c}")
                        nc.sync.dma_start(sfd[:], sdnL[dt, :, ts(c, D)])
                        v = wdf[:].rearrange("p j h -> p (j h)")[:, ts(c, D)]
                        nc.gpsimd.dma_start(v, wdnL[dt, :, ts(c, D)])
                        nc.vector.tensor_scalar_add(v, v, -7.5)
                        nc.vector.tensor_tensor(v, v, sfd[:], mybir.AluOpType.mult)
                    return wdf

                dpipe = {0: dn_dequant(0)}
                for dt in range(DT):
                    if dt + 1 < DT:
                        dpipe[dt + 1] = dn_dequant(dt + 1)
                    wdf = dpipe.pop(dt)

                    for tt in range(NT):
                        ps = pp.tile([P, 512], F32, tag="mm")
                        for k in range(KH):
                            nc.tensor.matmul(ps[:], wdf[:, k, :],
                                             x2sb[:, k, ts(tt, 512)],
                                             start=(k == 0), stop=False)
                        nc.tensor.matmul(ps[:], b2p[:, ts(dt, P)], v1t[:, ts(tt, 512)],
                                         start=False, stop=True)
                        yo = yp.tile([P, 512], F32, tag="yo")
                        nc.scalar.copy(yo[:], ps[:])
                        nc.sync.dma_start(y2t[ts(dt, P), ts(tt, 512)], yo[:])

    nc.compile()
    return nc


_NC = None


def build_in_maps(inputs):
    x1 = np.ascontiguousarray(np.asarray(inputs["x1"], dtype=np.float32))
    B, S, _ = x1.shape
    xf = x1.reshape(B * S, D)

    wq = np.asarray(inputs["w_up_q"], dtype=np.int32)        # [H, D]
    sup = np.asarray(inputs["w_up_scale"], dtype=np.float32)  # [H, 32]
    # codes, lhsT-tiled: wupL[k][p, j*128+h] = w_up_q[k*128+h, j*128+p]
    wupL = np.ascontiguousarray(
        wq.T.reshape(KD, P, KH, P).transpose(2, 1, 0, 3).reshape(KH, P, D))
    # scales expanded to the same layout (replication only, f16)
    sfull = np.repeat(sup.T, 64, axis=0)                      # [D, H]
    supL = np.ascontiguousarray(
        sfull.reshape(KD, P, KH, P).transpose(2, 1, 0, 3).reshape(KH, P, D)
    ).astype(np.float16)

    wdq = np.asarray(inputs["w_down_q"], dtype=np.int32)      # [D, H]
    sdn = np.asarray(inputs["w_down_scale"], dtype=np.float32)  # [D, 128]
    # codes, lhsT-tiled: wdnL[dt][p, k*128+d] = w_down_q[dt*128+d, k*128+p]
    wdnL = np.ascontiguousarray(
        wdq.T.reshape(KH, P, DT, P).transpose(2, 1, 0, 3).reshape(DT, P, H))
    sdfull = np.repeat(sdn.T, 64, axis=0)                     # [H, D]
    sdnL = np.ascontiguousarray(
        sdfull.reshape(KH, P, DT, P).transpose(2, 1, 0, 3).reshape(DT, P, H)
    ).astype(np.float16)

    shared = {
        "wupL": wupL, "supL": supL,
        "bup": np.ascontiguousarray(np.asarray(inputs["b_up"], dtype=np.float32)),
        "a1": np.ascontiguousarray(np.asarray(inputs["w_up_lora_a"], dtype=np.float32)),
        "b1": np.ascontiguousarray(np.asarray(inputs["w_up_lora_b"], dtype=np.float32)),
        "wdnL": wdnL, "sdnL": sdnL,
        "bdn": np.ascontiguousarray(np.asarray(inputs["b_down"], dtype=np.float32)),
        "a2": np.ascontiguousarray(np.asarray(inputs["w_down_lora_a"], dtype=np.float32)),
        "b2": np.ascontiguousarray(np.asarray(inputs["w_down_lora_b"], dtype=np.float32)),
    }
    return [{"x1c": np.ascontiguousarray(xf[c * T:(c + 1) * T]), **shared}
            for c in range(NCORES)]


def kernel(x1, w_up_q, w_up_scale, b_up, w_up_lora_a, w_up_lora_b,
           w_down_q, w_down_scale, b_down, w_down_lora_a, w_down_lora_b):
    global _NC, LAST_RESULTS
    if _NC is None:
        _NC = _build()

    inputs = dict(x1=x1, w_up_q=w_up_q, w_up_scale=w_up_scale, b_up=b_up,
                  w_up_lora_a=w_up_lora_a, w_up_lora_b=w_up_lora_b,
                  w_down_q=w_down_q, w_down_scale=w_down_scale, b_down=b_down,
                  w_down_lora_a=w_down_lora_a, w_down_lora_b=w_down_lora_b)
    in_maps = build_in_maps(inputs)
    res = run_bass_kernel_spmd(_NC, in_maps, core_ids=list(range(NCORES)),
                               trace=TRACE)
    LAST_RESULTS = res
    B, S, _ = np.asarray(x1).shape
    # y2t is [D, T] per core — un-transpose on the host while gathering
    out = np.concatenate([res.results[c]["y2t"].T for c in range(NCORES)], axis=0)
    return np.ascontiguousarray(out).reshape(B, S, D)
